# revision 77
# baseline (speedup 1.0000x reference)
"""Trainium2 Bass kernel: Longformer-style windowed attention with rotary,
head-averaged K/V (step_attn), fused QKV/out projections.

Sharding: 8 cores = (batch 2) x (sequence-quarter 4). Each core computes its
512 output rows for all 16 heads. No collectives: the windowed attention for
a 512-row quarter only needs 6 key-tiles (128 rows each) of the head-averaged
K/V plus the 64 global-token rows, all of which the core computes itself from
host-sliced hidden-state rows. Head-averaging of K/V commutes with rotary and
with the (linear) projection, so the K/V-mean projection weights are folded on
host to [2048, 256].

fp8 strategy (DoubleRow matmuls, 2 k-tiles of 128 per instruction at 0.5
cycles/row): error-attenuated paths (Q proj, K proj, QK scores) run naive fp8;
the V projection is hi+lo compensated (residual term via one DoubleRow matmul
per k-tile pairing hid_lo*w_hi + hi*lo); attention-value matmul, softmax sums
and out-projection stay bf16. Weights are host-scaled by 64 to clear the fp8
subnormal range; the 1/64 folds into downstream constant scales.
"""

import sys

for _p in ("/opt/trn_rl_repo", "/root/.axon_site/_ro/trn_rl_repo"):
    if _p not in sys.path:
        sys.path.append(_p)

import numpy as np
import ml_dtypes

import concourse.bass as bass
import concourse.tile as tile
from concourse import bacc
from concourse import bass_isa
import concourse.mybir as mybir
from concourse.bass_utils import run_bass_kernel_spmd

F32 = mybir.dt.float32
BF16 = mybir.dt.bfloat16
F8 = mybir.dt.float8e4
F8NP = ml_dtypes.float8_e4m3
DR = mybir.MatmulPerfMode.DoubleRow
MUL = mybir.AluOpType.mult
ADD = mybir.AluOpType.add
SUB = mybir.AluOpType.subtract
DIV = mybir.AluOpType.divide
COPY = mybir.ActivationFunctionType.Copy
EXP = mybir.ActivationFunctionType.Exp

H = 16
D = 128
ROT = 32
HALF = 16  # ROT // 2
WIN = 256
G = 64
BASE = 10000.0
S = 2048
HD = H * D
B = 2
NCORES = 8
QROWS = 512          # rows per core
NKV = 6              # kv key-tiles per core
KVG_ROWS = NKV * 128 + G  # 832
SCALE = 1.0 / float(np.sqrt(np.float32(D)))
WS = 64.0            # host weight scale (fp8 subnormal avoidance)


# ---------------------------------------------------------------- device ----

def build_nc():
    nc = bacc.Bacc("TRN2", target_bir_lowering=False, debug=False,
                   num_devices=NCORES)

    aps = {}
    def inp(name, shape, dt):
        aps[name] = nc.dram_tensor(name, shape, dt, kind="ExternalInput").ap()

    # hidT8: transposed hidden states, fp8, planes (lo, hi), kv-tile-major so
    # each tile is one contiguous DMA; hidg8 holds the global-token rows
    inp("hidT8", [NKV, 128, 16, 2, 128], F8)
    inp("hidg8", [128, 16, 2, G], F8)
    inp("wq8", [128, 16, HD], F8)            # 64*Wq, feature-major, naive
    inp("wkv8", [128, 16, 2, 2 * D], F8)     # 64*Wkv, planes (hi, lo16)
    inp("wo", [HD, HD], BF16)
    inp("bqb", [128, HD], BF16)              # 64*b_q broadcast to partitions
    inp("bob", [128, HD], BF16)              # b_o broadcast to partitions
    inp("pk128", [128, 8 * HALF + 2 * NKV * HALF + NKV], F32)
    inp("pk64", [G, 2 * HALF], F32)
    inp("pkb", [1, 2 * D + HD], BF16)        # 64*b_kv | b_o
    inp("i8", [128, 128], F8)                # 240 * identity (mask carrier)
    inp("ib16", [128, 128], BF16)            # identity (PE transposes)
    inp("mask8", [128, 16, 512], F8)         # -240*!valid: (3 win t + glob)x4h
    aps["out"] = nc.dram_tensor("out", [QROWS, HD], BF16,
                                kind="ExternalOutput").ap()

    with tile.TileContext(nc) as tc:
        _build_tile(nc, tc, aps)
    nc.compile()
    return nc


def _build_tile(nc, tc, aps):
    from contextlib import ExitStack
    import os
    ctx = ExitStack()
    _PH = int(os.environ.get("KERNEL_PHASES", "4"))

    persist = ctx.enter_context(tc.tile_pool(name="persist", bufs=1))
    ps = ctx.enter_context(tc.tile_pool(name="ps", bufs=8, space="PSUM"))
    # right-side pools: hidden states + streamed wq chunks + evac temps (live
    # through the interleaved projection work inside early attention steps)
    ctxR = ExitStack()
    hidp = ctxR.enter_context(tc.tile_pool(name="hidp", bufs=1, side="right"))
    wpool = ctxR.enter_context(tc.tile_pool(name="wstream", bufs=16, side="right"))
    epool = ctxR.enter_context(tc.tile_pool(name="evac", bufs=2, side="right"))

    # ---------------- persistent tiles
    # hidT8 SBUF layout is kv-tile-major (like the DRAM layout) so each
    # per-tile DMA writes >=4KB contiguous runs (full DMA bus rate); slot 6
    # holds the global-token rows in its first 64 columns
    hidT8 = hidp.tile([128, NKV + 1, 16, 2, 128], F8, tag="hidT8")
    bqb = hidp.tile([128, HD], BF16, tag="bqb")
    bob = persist.tile([128, HD], BF16, tag="bob")
    q_sb = persist.tile([128, 4, HD], BF16, tag="q_sb")
    # q8: fp8 q (16 head blocks) + 16 additive-mask blocks (DoubleRow halves)
    q8 = persist.tile([128, 32, QROWS], F8, tag="q8")
    kv_sb = persist.tile([128, NKV, 2 * D], BF16, tag="kv_sb")
    kvg_sb = persist.tile([G, 2 * D], BF16, tag="kvg_sb")
    kTm = persist.tile([128, NKV, 2, 128], F8, tag="kTm")
    kgTm = persist.tile([128, 2, G], F8, tag="kgTm")
    wkv8 = persist.tile([128, 16, 2, 2 * D], F8, tag="wkv8")
    wo_sb = persist.tile([128, H, HD], BF16, tag="wo_sb")
    i8_sb = persist.tile([128, 128], F8, tag="i8")
    ib16 = persist.tile([128, 128], BF16, tag="ib16")
    ones_c64 = persist.tile([128, 1], BF16, tag="ones_c64")  # 64.0 column
    ones_r = persist.tile([1, 128], BF16, tag="ones_r")   # row (K=1, M=128)
    pk128 = persist.tile([128, 8 * HALF + 2 * NKV * HALF + NKV], F32,
                         tag="pk128")
    pk64 = persist.tile([G, 2 * HALF], F32, tag="pk64")
    pkb = persist.tile([1, 2 * D + HD], BF16, tag="pkb")
    cq_sb = pk128[:, 0:64].rearrange("p (so r) -> p so r", r=HALF)
    sq_sb = pk128[:, 64:128].rearrange("p (so r) -> p so r", r=HALF)
    ckv_sb = pk128[:, 128:224].rearrange("p (t r) -> p t r", r=HALF)
    skv_sb = pk128[:, 224:320].rearrange("p (t r) -> p t r", r=HALF)
    am_sb = pk128[:, 320:326]
    cg_sb = pk64[:, 0:HALF]
    sg_sb = pk64[:, HALF:2 * HALF]
    bkv_sb = pkb[:, 0:2 * D]
    bo_sb = pkb[:, 2 * D:2 * D + HD]

    # ---------------- small loads (Activation HWDGE queue, ordered by need)
    # 64.0 so the z broadcast yields rzb = 1/(64z), cancelling po's 64-scale
    # in the aT multiply
    nc.gpsimd.memset(ones_c64[:], 64.0)
    nc.gpsimd.memset(ones_r[:], 1.0)
    for nm, t in (("pk128", pk128), ("pk64", pk64), ("pkb", pkb),
                  ("ib16", ib16), ("i8", i8_sb), ("bqb", bqb)):
        nc.scalar.dma_start(out=t[:], in_=aps[nm])
    # identity carriers into the DoubleRow second halves of kTm / kgTm
    nc.vector.tensor_copy(
        kTm[:, :, 1, :],
        i8_sb[:].rearrange("p (o d) -> p o d", o=1).to_broadcast([128, NKV, 128]))
    nc.vector.tensor_copy(kgTm[:, 1, :], i8_sb[:, 0:G])

    # ---------------- bulk DMAs (sync HWDGE queue, priority order). All
    # transfers serialize on the DMA engine pool, so the order below IS the
    # startup schedule: wkv + kv tiles 0-2+glob + the masks + wq head group
    # 0 gate the attention start; the rest streams underneath it.
    def load_hid(st):
        if st == "g":
            nc.sync.dma_start(out=hidT8[:, NKV, :, :, 0:G], in_=aps["hidg8"])
        else:
            nc.sync.dma_start(out=hidT8[:, st], in_=aps["hidT8"][st])

    wq_ts = {}
    def load_wq_hg(hg):
        for k2 in range(8):
            t = wpool.tile([128, 2, 512], F8, tag="wq_t", name=f"wq{hg}_{k2}")
            nc.sync.dma_start(
                out=t[:],
                in_=aps["wq8"][:, 2 * k2:2 * k2 + 2, hg * 512:(hg + 1) * 512])
            wq_ts[(k2, hg)] = t

    def load_wo(cn):
        nc.sync.dma_start(
            out=wo_sb[:, :, cn * 512:(cn + 1) * 512],
            in_=aps["wo"].rearrange("(h p) n -> p h n", p=128)
            [:, :, cn * 512:(cn + 1) * 512])

    nc.sync.dma_start(out=wkv8[:], in_=aps["wkv8"])
    for st in (0, 1, 2, "g"):
        load_hid(st)
    # additive mask blocks (g = 4..7 of q8's 8 groups)
    nc.sync.dma_start(out=q8[:, 16:32, :], in_=aps["mask8"])
    load_wq_hg(0)
    for st in (3, 4, 5):
        load_hid(st)
    load_wq_hg(1)
    load_wq_hg(2)
    load_wq_hg(3)
    nc.scalar.dma_start(out=bob[:], in_=aps["bob"])
    load_wo(0)
    load_wo(1)
    load_wo(2)
    load_wo(3)

    # rotary (in-place, f32 temps): x1' = x1*c - x2*s ; x2' = x2*c + x1*s
    def rotary(x1, x2, c, s, shape, tag, eng=None):
        eng = eng or nc.vector
        t1 = epool.tile(shape, F32, tag=tag + "1")
        t2 = epool.tile(shape, F32, tag=tag + "2")
        eng.tensor_tensor(out=t1[:], in0=x1, in1=s, op=MUL)
        eng.tensor_tensor(out=t2[:], in0=x2, in1=s, op=MUL)
        eng.tensor_tensor(out=x1, in0=x1, in1=c, op=MUL)
        eng.tensor_tensor(out=x1, in0=x1, in1=t2[:], op=SUB)
        eng.tensor_tensor(out=x2, in0=x2, in1=c, op=MUL)
        eng.tensor_tensor(out=x2, in0=x2, in1=t1[:], op=ADD)

    if _PH < 2:
        ctxR.close()
        ctx.close()
        return

    # ---------------- kv projection, two stages per tile: kv_mm (main +
    # corr matmuls, evac, k rotary) and kv_tp (PE transpose into the fp8 kT
    # layout) so the rotary latency hides behind other PE work
    def kv_mm(st):
        m = 128 if st != "g" else G
        sti = NKV if st == "g" else st
        pkv = ps.tile([128, 512], F32, tag="ps", name=f"pkv{st}")
        pkc = ps.tile([128, 512], F32, tag="ps", name=f"pkc{st}")
        for k2 in range(8):
            nc.tensor.matmul(pkv[:m, :2 * D],
                             hidT8[:, sti, 2 * k2:2 * k2 + 2, 1, 0:m],
                             wkv8[:, 2 * k2:2 * k2 + 2, 0, :],
                             start=(k2 == 0), stop=False, perf_mode=DR)
        nc.tensor.matmul(pkv[:m, :2 * D], ones_r[:, :m], bkv_sb[:],
                         start=False, stop=True)
        for kt in range(16):
            nc.tensor.matmul(pkc[:m, :D],
                             hidT8[:, sti, kt, :, 0:m],
                             wkv8[:, kt, :, D:2 * D],
                             start=(kt == 0), stop=(kt == 15), perf_mode=DR)
        dst = kvg_sb[:] if st == "g" else kv_sb[:, st, :]
        vcorr = epool.tile([128, D], F32, tag="vcorr")
        nc.scalar.activation(vcorr[:m], pkc[:m, :D], COPY, scale=1.0 / 16.0)
        nc.scalar.copy(dst[:m, 0:D], pkv[:m, 0:D])
        nc.vector.tensor_tensor(out=dst[:m, D:2 * D], in0=pkv[:m, D:2 * D],
                                in1=vcorr[:m], op=ADD)
        if st == "g":
            rotary(kvg_sb[:, 0:HALF], kvg_sb[:, HALF:2 * HALF],
                   cg_sb[:], sg_sb[:], [G, HALF], "rg", eng=nc.gpsimd)
        else:
            rotary(kv_sb[:, st:st + 1, 0:HALF],
                   kv_sb[:, st:st + 1, HALF:2 * HALF],
                   ckv_sb[:, st:st + 1, :], skv_sb[:, st:st + 1, :],
                   [128, 1, HALF], "rkv", eng=nc.gpsimd)

    def kv_tp(st):
        if st == "g":
            ptgf = ps.tile([128, 128], BF16, tag="ps", name="ptg")
            nc.tensor.transpose(ptgf[:, 0:G], kvg_sb[:, 0:128],
                                ib16[0:G, 0:G])
            nc.scalar.activation(kgTm[:, 0, :], ptgf[:, 0:G], COPY,
                                 scale=1.0 / WS)
        else:
            ptkf = ps.tile([128, 128], BF16, tag="ps", name=f"ptk{st}")
            nc.tensor.transpose(ptkf[:], kv_sb[:, st, 0:128], ib16[:])
            nc.scalar.activation(kTm[:, st, 0, :], ptkf[:], COPY,
                                 scale=1.0 / WS)

    # ---------------- q projection, two stages per (head-group, so) chunk:
    # q_mm (matmuls + bias evac + rotary in row-major layout) and q_tp
    # (PE transposes + fp8 quant into q8), pipelined one fill window apart
    def q_mm(hg, so):
        pq = ps.tile([128, 512], F32, tag="ps", name=f"pq{hg}_{so}")
        for k2 in range(8):
            nc.tensor.matmul(pq[:],
                             hidT8[:, 2 + so, 2 * k2:2 * k2 + 2, 1, :],
                             wq_ts[(k2, hg)][:],
                             start=(k2 == 0), stop=(k2 == 7), perf_mode=DR)
        cols = slice(hg * 512, (hg + 1) * 512)
        nc.vector.tensor_tensor(out=q_sb[:, so, cols], in0=pq[:],
                                in1=bqb[:, cols], op=ADD)
        qv = q_sb[:, so, cols].rearrange("p (h d) -> p h d", d=D)
        c = cq_sb[:, so:so + 1, :].to_broadcast([128, 4, HALF])
        s = sq_sb[:, so:so + 1, :].to_broadcast([128, 4, HALF])
        rotary(qv[:, :, 0:HALF], qv[:, :, HALF:2 * HALF], c, s,
               [128, 4, HALF], "rq", eng=nc.gpsimd)

    def q_tp(hg, so):
        ptf = ps.tile([128, 4, 128], BF16, tag="ps", name=f"ptq{hg}_{so}")
        for h in range(4):
            nc.tensor.transpose(
                ptf[:, h, :],
                q_sb[:, so, (4 * hg + h) * 128:(4 * hg + h + 1) * 128],
                ib16[:])
        if (hg + so) % 2 == 0:
            nc.scalar.activation(
                q8[:, 4 * hg:4 * hg + 4, so * 128:(so + 1) * 128],
                ptf[:], COPY, scale=1.0 / WS)
        else:
            nc.vector.tensor_scalar_mul(
                q8[:, 4 * hg:4 * hg + 4, so * 128:(so + 1) * 128],
                ptf[:], 1.0 / WS)

    # pre-attention: kv for the L0 window + all of head group 0 (so its wq
    # stream buffers recycle immediately) + the first chunk of head group 1
    for st in (0, 1, 2, "g"):
        kv_mm(st)
    q_mm(0, 0)
    for st in (0, 1, 2, "g"):
        kv_tp(st)
    for so in range(1, 4):
        q_mm(0, so)
    q_mm(1, 0)
    for so in range(4):
        q_tp(0, so)
    q_tp(1, 0)

    # ---------------- fused attention + out-projection, per block L
    wexp = ctx.enter_context(tc.tile_pool(name="wexp", bufs=8))
    rzp = ctx.enter_context(tc.tile_pool(name="rzp", bufs=4))
    opool = ctx.enter_context(tc.tile_pool(name="opool", bufs=1))
    if _PH < 3:
        ctx.close()
        return
    aT_tiles = [None] * 4
    aT8_tiles = [None] * 4
    # g-blocks of q8: 0-3 = q head groups, 4-6 = win masks t, 7 = glob mask
    q8v = q8[:].rearrange("p (g h) r -> p g h r", h=4)

    def scores(L, hg):
        """Emit the 4 DoubleRow score matmuls (half1 carries additive mask)."""
        st = {}
        for t in range(3):
            rhs = q8v[:, hg:5 + t:(4 + t - hg), :, L * 128:(L + 1) * 128]
            p_t = ps.tile([128, 512], F32, tag="ps", name=f"p_t{L}_{hg}_{t}")
            nc.tensor.matmul(p_t[:], kTm[:, L + t, :, :], rhs,
                             start=True, stop=True, perf_mode=DR)
            st[t] = p_t
        rhs = q8v[:, hg:8:(7 - hg), :, L * 128:(L + 1) * 128]
        p_g = ps.tile([128, 512], F32, tag="ps", name=f"p_g{L}_{hg}")
        nc.tensor.matmul(p_g[:G, :], kgTm[:], rhs, start=True, stop=True,
                         perf_mode=DR)
        st["g"] = p_g
        return st

    def exps(L, hg, st):
        """exp tiles + bf16 pre-sum for the softmax denominator (one step
        ahead of the pz/AV consumption)."""
        w_t = []
        for t in range(3):
            w = wexp.tile([128, 512], BF16, tag="wexp", bufs=8, name=f"w{L}_{hg}_{t}")
            nc.scalar.activation(w[:], st[t][:], EXP,
                                 bias=am_sb[:, L + t:L + t + 1], scale=SCALE)
            w_t.append(w)
        w_g = wexp.tile([G, 512], BF16, tag="wexpg", bufs=4,
                        name=f"wg{L}_{hg}")
        nc.scalar.activation(w_g[:], st["g"][:G, :], EXP, scale=SCALE)
        wsum = wexp.tile([128, 512], BF16, tag="wsum", bufs=3,
                         name=f"wsum{L}_{hg}")
        with nc.allow_low_precision(reason="softmax denom: bf16 partial sums "
                                    "match the baseline's bf16 z quantization"):
            nc.vector.tensor_tensor(out=wsum[:], in0=w_t[0][:], in1=w_t[1][:],
                                    op=ADD)
            nc.vector.tensor_tensor(out=wsum[:], in0=wsum[:], in1=w_t[2][:],
                                    op=ADD)
            nc.vector.tensor_tensor(out=wsum[:G, :], in0=wsum[:G, :],
                                    in1=w_g[:], op=ADD)
        return w_t, w_g, wsum

    def denom(L, hg, wsum):
        """8z broadcast to all partitions via one all-8s ones-matmul, then
        a single reciprocal into SBUF."""
        pz = ps.tile([128, 512], F32, tag="ps", name=f"pz{L}_{hg}")
        nc.tensor.matmul(pz[:], ones_c64[:].to_broadcast([128, 128]), wsum[:],
                         start=True, stop=True)
        rzb = rzp.tile([128, 512], BF16, tag="rzb", bufs=3,
                       name=f"rzb{L}_{hg}")
        with nc.allow_low_precision(reason="1/z in bf16 matches the "
                                    "baseline's bf16 z quantization"):
            nc.vector.reciprocal(out=rzb[:], in_=pz[:])
        return rzb

    def att_v(L, hg, w_t, w_g, rzb, aT):
        """AV matmuls + normalized aT for (L, hg)."""
        po = ps.tile([128, 512], F32, tag="ps", name=f"po{L}_{hg}")
        for t in range(3):
            nc.tensor.matmul(po[:], kv_sb[:, L + t, D:2 * D], w_t[t][:],
                             start=(t == 0), stop=False)
        nc.tensor.matmul(po[:], kvg_sb[:, D:2 * D], w_g[:],
                         start=False, stop=True)
        nc.vector.tensor_tensor(
            out=aT[:, 4 * hg:4 * hg + 4, :],
            in0=po[:].rearrange("p (h s) -> p h s", s=128),
            in1=rzb[:].rearrange("p (h s) -> p h s", s=128),
            op=MUL)

    # out-projection for block Lp, one output-column chunk ncn per call
    oproj_state = {}

    def oproj_seg(Lp, ncn):
        aT = aT_tiles[Lp]
        cols = slice(ncn * 512, (ncn + 1) * 512)
        po2 = ps.tile([128, 512], F32, tag="ps", name=f"po2_{Lp}_{ncn}")
        for h in range(16):
            nc.tensor.matmul(po2[:], aT[:, h, :], wo_sb[:, h, cols],
                             start=(h == 0), stop=(h == 15))
        o_sb = oproj_state.setdefault(
            Lp, opool.tile([128, HD], BF16, tag="o_sb", name=f"o_sb{Lp}"))
        nc.vector.tensor_tensor(out=o_sb[:, cols], in0=po2[:],
                                in1=bob[:, cols], op=ADD)
        if ncn == 3:
            nc.sync.dma_start(out=aps["out"][Lp * 128:(Lp + 1) * 128, :],
                              in_=o_sb[:])

    # software pipeline over the 16 (L, hg) steps: scores+exps for step n+1
    # run one step ahead; the denominator chain (pz -> reciprocal) is issued
    # early so its latency hides under the interleaved fill work (remaining
    # q chunks + kv tiles 3-5 during L0, out-proj segments of block L-1
    # afterwards); AV + the aT multiply close each step.
    steps = [(L, hg) for L in range(4) for hg in range(4)]
    # fill schedule: each head group completes all four so-quarters in one
    # window, ahead of the scores that consume it, so its wq stream buffers
    # recycle for the next head group's DMA (wpool holds two head groups);
    # kv tiles 3-5 land before scores(1..3, *) need them. Transpose stages
    # run one window after their matmul stage (rotary latency hiding).
    fills = {0: [(q_mm, 1, 1), (q_mm, 1, 2), (q_mm, 1, 3), (q_mm, 2, 0)],
             1: [(kv_mm, 3), (q_mm, 2, 1), (q_mm, 2, 2), (q_mm, 2, 3),
                 (q_mm, 3, 0),
                 (q_tp, 1, 1), (q_tp, 1, 2), (q_tp, 1, 3), (q_tp, 2, 0)],
             2: [(kv_mm, 4), (q_mm, 3, 1), (q_mm, 3, 2), (q_mm, 3, 3),
                 (kv_tp, 3), (q_tp, 2, 1), (q_tp, 2, 2), (q_tp, 2, 3),
                 (q_tp, 3, 0)],
             3: [(kv_mm, 5), (kv_tp, 4),
                 (q_tp, 3, 1), (q_tp, 3, 2), (q_tp, 3, 3)],
             4: [(kv_tp, 5)]}
    for L in range(4):
        aT_tiles[L] = wexp.tile([128, H, 128], BF16, tag="aT", bufs=3,
                                name=f"aT{L}")
    wx = {0: exps(0, 0, scores(0, 0))}
    for n, (L, hg) in enumerate(steps):
        w_t, w_g, wsum = wx.pop(n)
        rzb = denom(L, hg, wsum)
        for f in fills.get(n, ()):
            f[0](*f[1:])
        if n + 1 < len(steps):
            Ln, hgn = steps[n + 1]
            wx[n + 1] = exps(Ln, hgn, scores(Ln, hgn))
        if _PH >= 4 and L >= 1:
            oproj_seg(L - 1, hg)
        att_v(L, hg, w_t, w_g, rzb, aT_tiles[L])
    if _PH >= 4:
        for ncn in range(4):
            oproj_seg(3, ncn)

    ctxR.close()
    ctx.close()


# ------------------------------------------------------------------ host ----

_NC_CACHE = None


def _get_nc():
    global _NC_CACHE
    if _NC_CACHE is None:
        _NC_CACHE = build_nc()
    return _NC_CACHE


def _f8(x):
    return np.asarray(x, np.float32).astype(F8NP)


def _f8_pair(x):
    """(hi, lo) with x ~= hi + lo/16, both fp8."""
    hi = _f8(x)
    lo = _f8((np.asarray(x, np.float32) - hi.astype(np.float32)) * 16.0)
    return hi, lo


def make_in_maps(hidden_states, attention_mask, glob_idx, W_qkv, b_qkv, W_o, b_o):
    bf = ml_dtypes.bfloat16
    hidden_states = np.asarray(hidden_states, np.float32)
    attention_mask = np.asarray(attention_mask, np.float32)
    glob_idx = np.asarray(glob_idx)
    W_qkv = np.asarray(W_qkv, np.float32)
    b_qkv = np.asarray(b_qkv, np.float32)
    W_o = np.asarray(W_o, np.float32)
    b_o = np.asarray(b_o, np.float32)

    w3 = W_qkv.reshape(HD, H, 3 * D)
    wq = np.ascontiguousarray(w3[:, :, :D].reshape(HD, HD))
    wkv = np.concatenate([w3[:, :, D:2 * D].mean(axis=1),
                          w3[:, :, 2 * D:].mean(axis=1)], axis=1)
    # feature-major fp8 layouts, 64-scaled
    wq8 = _f8((WS * wq).reshape(16, 128, HD).transpose(1, 0, 2))
    wkv_hi, wkv_lo = _f8_pair((WS * wkv).reshape(16, 128, 2 * D))
    wkv8 = np.stack([wkv_hi, wkv_lo], axis=2).transpose(1, 0, 2, 3)
    wkv8 = np.ascontiguousarray(wkv8)      # [128, 16, 2(hi,lo), 256]

    b3 = b_qkv.reshape(H, 3 * D)
    bq = np.ascontiguousarray(b3[:, :D].reshape(1, HD)).astype(np.float32)
    bqb = np.ascontiguousarray(np.broadcast_to(WS * bq, (128, HD))).astype(bf)
    bob = np.ascontiguousarray(np.broadcast_to(b_o[None, :], (128, HD))
                               ).astype(bf)
    bkv = np.concatenate([b3[:, D:2 * D].mean(axis=0),
                          b3[:, 2 * D:].mean(axis=0)])[None, :] * WS
    bo = b_o[None, :]
    pkb = np.concatenate([bkv, bo], axis=1).astype(bf)
    wo = W_o.astype(bf)

    inv_freq = 1.0 / (BASE ** (np.arange(0, ROT, 2, dtype=np.float32) / ROT))
    freqs = np.arange(S, dtype=np.float32)[:, None] * inv_freq[None, :]  # [S,16]
    cos_all = np.cos(freqs).astype(np.float32)
    sin_all = np.sin(freqs).astype(np.float32)

    in_maps = []
    for c in range(NCORES):
        b, q = divmod(c, 4)
        t0 = 4 * q - 2
        tiles = [max(0, t0 + i) for i in range(NKV)]       # clipped content
        intended = [t0 + i for i in range(NKV)]
        kv_rows = np.concatenate([np.arange(t * 128, t * 128 + 128)
                                  for t in tiles])
        g_rows = glob_idx[b].astype(np.int64)
        rows = np.concatenate([kv_rows, g_rows])
        hid_c = np.ascontiguousarray(hidden_states[b][rows])   # [832, 2048]
        # transpose to [128 fsub, 16 ftile, rows], fp8 (lo, hi) planes,
        # then kv-tile-major so each tile is one contiguous DMA
        hidT = hid_c.T.reshape(16, 128, KVG_ROWS).transpose(1, 0, 2)
        h_hi, h_lo = _f8_pair(hidT)
        hid8 = np.stack([h_lo, h_hi], axis=2)       # [128, 16, 2, 832]
        hidT8 = np.ascontiguousarray(
            np.stack([hid8[:, :, :, st * 128:(st + 1) * 128]
                      for st in range(NKV)], axis=0))
        hidg8 = np.ascontiguousarray(hid8[:, :, :, NKV * 128:KVG_ROWS])

        q_rows = np.arange(QROWS * q, QROWS * (q + 1))
        cos_q = cos_all[q_rows].reshape(4, 128, HALF).transpose(1, 0, 2).copy()
        sin_q = sin_all[q_rows].reshape(4, 128, HALF).transpose(1, 0, 2).copy()
        cos_kv = cos_all[kv_rows].reshape(NKV, 128, HALF).transpose(1, 0, 2).copy()
        sin_kv = sin_all[kv_rows].reshape(NKV, 128, HALF).transpose(1, 0, 2).copy()
        cos_g = cos_all[g_rows].copy()
        sin_g = sin_all[g_rows].copy()

        am = attention_mask[b, 0, 0]                        # [S]
        am_loc = am[kv_rows].reshape(NKV, 128).T.copy()     # [128, NKV]

        # additive fp8 masks, folded into the DoubleRow score matmuls:
        # score += 240 * m8 where m8 = -240 for invalid -> -57600 pre-scale.
        # layout [key-p, block(t0,t1,t2,glob), 4h replicated, 4L*128 rows]
        mask8 = np.full((128, 4, 4, 512), -240.0, np.float32)
        for L in range(4):
            rows_glb = QROWS * q + L * 128 + np.arange(128)
            for t in range(3):
                it = intended[L + t]
                if it < 0:
                    continue
                key_pos = it * 128 + np.arange(128)
                valid = (key_pos[:, None] <= rows_glb[None, :]) & \
                        (key_pos[:, None] >= rows_glb[None, :] - (WIN - 1))
                mask8[:, t, :, L * 128:(L + 1) * 128] = \
                    np.where(valid, 0.0, -240.0)[:, None, :]
        # glob (block 3): row >= WIN and glob_idx < row - WIN; key rows 64-127
        # are dead (identity carrier is zero there)
        rows_glb = QROWS * q + np.arange(QROWS)
        validg = ((rows_glb[None, :] >= WIN) &
                  (g_rows[:, None] < rows_glb[None, :] - WIN))
        mask8[:G, 3, :, :] = np.where(validg, 0.0, -240.0)[:, None, :]
        mask8 = mask8.reshape(128, 16, 512)

        i8 = (240.0 * np.eye(128, dtype=np.float32))
        ib16 = np.eye(128, dtype=np.float32).astype(bf)

        pk128 = np.concatenate(
            [cos_q.reshape(128, 64), sin_q.reshape(128, 64),
             cos_kv.reshape(128, 96), sin_kv.reshape(128, 96),
             am_loc], axis=1).astype(np.float32)
        pk64 = np.concatenate([cos_g, sin_g], axis=1).astype(np.float32)
        in_maps.append({
            "hidT8": hidT8, "hidg8": hidg8,
            "wq8": wq8, "wkv8": wkv8, "wo": wo,
            "bqb": bqb, "bob": bob,
            "pk128": pk128, "pk64": pk64, "pkb": pkb,
            "i8": _f8(i8), "ib16": ib16, "mask8": _f8(mask8),
        })
    return in_maps


def kernel(hidden_states, attention_mask, glob_idx, W_qkv, b_qkv, W_o, b_o):
    nc = _get_nc()
    in_maps = make_in_maps(hidden_states, attention_mask, glob_idx,
                           W_qkv, b_qkv, W_o, b_o)
    res = run_bass_kernel_spmd(nc, in_maps, core_ids=list(range(NCORES)))
    out = np.empty((B, S, HD), np.float32)
    for c in range(NCORES):
        b, q = divmod(c, 4)
        out[b, QROWS * q:QROWS * (q + 1), :] = \
            res.results[c]["out"].astype(np.float32)
    return out

